# revision 13
# baseline (speedup 1.0000x reference)
"""Bass/Tile kernel for nn_MicrotubuleAttention on 8 Trainium2 NeuronCores.

Math: the reference adds (1 - gtp) * NEG (NEG = -1e9) to every causal
off-diagonal score. With gamma clipped to >= 1e-4, the smallest penalty is
-1e9 * (1 - exp(-1e-4)) ~= -1e5, so after float32 softmax (max-subtract +
exp) every off-diagonal weight underflows to exactly 0 and attention is
exactly the identity. Hence:

    out = repeat_gqa(x @ Wv) @ Wo = (x @ Wv) @ Wo_folded

where Wo_folded[c*64+d, :] = sum_r Wo[(4c+r)*64+d, :] sums the 4 query-head
row blocks that share KV head c. Q/K/RoPE/polarity/gamma provably do not
affect the f32 output (verified ~1e-6 max rel err against the jax reference).

Sharding: data parallel over rows. B*T = 4096 rows split 8 ways -> 512 rows
per core; Wv/Wo broadcast. Per core:
  1. transpose the x-shard via PE (fp32 has no DMA transpose): xT = x^T
  2. stage 1:  vT[j', m] = Wv[:, j']^T @ x^T   (j' = 256 folded channels)
  3. fold Wo entirely in the DMA path (gpsimd accum-DMAs) -> WoF [256, 1024]
  4. stage 2:  out[m, n] = vT^T @ WoF

DMA priority: x gates the PE pipeline start, Wv gates stage 1, Wo only
gates stage 2 - so Wo transfers are chained behind x+Wv via explicit dep
edges to keep them from stealing HBM bandwidth early.
"""

import os
import sys

import numpy as np

for _p in ("/opt/trn_rl_repo", "/opt/pypackages"):
    if os.path.isdir(_p) and _p not in sys.path:
        sys.path.append(_p)

B, T, D_MODEL = 2, 2048, 1024
H_Q, H_KV, D_HEAD = 16, 4, 64
N_CORES = 8
M_TOTAL = B * T              # 4096 rows
M_CORE = M_TOTAL // N_CORES  # 512 rows per core
P = 128
KK = D_MODEL // P            # 8 contraction chunks of 128
MC = M_CORE // P             # 4 row chunks of 128
NKV = H_KV * D_HEAD          # 256

TRACE = False          # test.py flips this to profile
TRACE_CORES = None
LAST_RESULTS = None    # BassKernelResults of the most recent run

_nc_cache = None


def _build_bass():
    import concourse.bass as bass
    import concourse.mybir as mybir
    import concourse.tile as tile
    from concourse import bacc
    from concourse.masks import make_identity
    from concourse.tile import add_dep_helper

    f32 = mybir.dt.float32
    ts = bass.ts

    nc = bacc.Bacc(None)
    x_d = nc.declare_dram_parameter("x", [M_CORE, D_MODEL], f32, isOutput=False)
    wv_d = nc.declare_dram_parameter("wv", [D_MODEL, NKV], f32, isOutput=False)
    wo_d = nc.declare_dram_parameter("wo", [H_Q * D_HEAD, D_MODEL], f32, isOutput=False)
    out_d = nc.declare_dram_parameter("out", [M_CORE, D_MODEL], f32, isOutput=True)

    with tile.TileContext(nc) as tc:
        with (
            tc.tile_pool(name="const", bufs=1) as const,
            tc.tile_pool(name="wo_pool", bufs=H_KV) as wo_pool,
            tc.tile_pool(name="x_pool", bufs=MC) as x_pool,
            tc.tile_pool(name="o_pool", bufs=2 * MC) as o_pool,
            tc.tile_pool(name="psum_t", bufs=2, space="PSUM") as psum_t,
            tc.tile_pool(name="psum_mm", bufs=2, space="PSUM") as psum_mm,
        ):
            # Keep every instruction to <=1 attached semaphore wait (this
            # walrus rejects more): all non-PE compute on the Vector engine;
            # a dummy transpose absorbs identity's gpsimd dep into PE's clock.
            identity = const.tile([P, P], f32)
            make_identity(nc, identity)
            warm = psum_t.tile([P, P], f32, tag="warm")
            nc.tensor.transpose(warm[:], identity[:], identity[:])

            # ---- transpose x-shard: xT[p, kk, m] = x[m, kk*128 + p] ----
            # 4 transposes share one PSUM bank -> one coarse DVE copyback per
            # bank instead of one per transpose (DVE was rate-limiting PE).
            xT = const.tile([P, KK, M_CORE], f32)
            x_dmas = []
            for mi in range(MC):
                x_sb = x_pool.tile([P, D_MODEL], f32, tag="x_in")
                x_dmas.append(nc.sync.dma_start(x_sb[:], x_d[ts(mi, P), :]))
                for g in range(2):
                    pt = psum_t.tile([P, 512], f32, tag="tp")
                    for j in range(4):
                        kk = g * 4 + j
                        nc.tensor.transpose(
                            pt[:, ts(j, P)], x_sb[:, ts(kk, P)], identity[:]
                        )
                    nc.vector.tensor_copy(
                        xT[:, ts(g, 4), ts(mi, P)],
                        pt.rearrange("p (j m) -> p j m", j=4),
                    )

            # ---- Wv: natural k-major layout  wv_sb[p, ko, n] ----
            wv_sb = const.tile([P, KK, NKV], f32)
            wv_dma = nc.sync.dma_start(
                wv_sb[:], wv_d.rearrange("(ko p) n -> p ko n", p=P)
            )
            add_dep_helper(wv_dma.ins, x_dmas[-1].ins, reason="x before wv")

            # ---- Wo: full GQA fold -> WoF[p, q, n], j' = q*128 + p ----
            # All folding happens in the DMA path (gpsimd accum-DMAs), no DVE:
            #   pair_c[p, n]  = Wo[256c + p, n] + Wo[256c + 128 + p, n]
            #   WoF[(c%2)*64 + d, c//2, n] = pair_c[d, n] + pair_c[64 + d, n]
            add = mybir.AluOpType.add
            wo_f = const.tile([P, 2, D_MODEL], f32)
            first_wo = None
            for c in range(H_KV):
                pair = wo_pool.tile([P, D_MODEL], f32, tag="wo_pair")
                d0 = nc.gpsimd.dma_start(pair[:], wo_d[256 * c : 256 * c + 128, :])
                if first_wo is None:
                    first_wo = d0
                nc.gpsimd.dma_start(
                    pair[:], wo_d[256 * c + 128 : 256 * c + 256, :], accum_op=add
                )
                lo = (c % 2) * 64
                dst = wo_f[lo : lo + 64, c // 2, :]
                nc.gpsimd.dma_start(dst, pair[0:64, :])
                nc.gpsimd.dma_start(dst, pair[64:128, :], accum_op=add)
            add_dep_helper(first_wo.ins, wv_dma.ins, reason="wv before wo")

            # ---- stage 1: vT[j', m] = sum_k Wv[k, j'] x[m, k] ----
            vT = const.tile([P, 2, M_CORE], f32)
            for q in range(2):
                ps = psum_mm.tile([P, M_CORE], f32, tag="s1")
                for kk in range(KK):
                    nc.tensor.matmul(
                        ps[:],
                        lhsT=wv_sb[:, kk, ts(q, P)],
                        rhs=xT[:, kk, :],
                        start=(kk == 0),
                        stop=(kk == KK - 1),
                    )
                nc.vector.tensor_copy(vT[:, q, :], ps[:])

            # ---- stage 2: out[m, n] = sum_j' vT[j', m] WoF[j', n] ----
            for mi in range(MC):
                for half in range(2):
                    ps = psum_mm.tile([P, 512], f32, tag="s2")
                    for q in range(2):
                        nc.tensor.matmul(
                            ps[:],
                            lhsT=vT[:, q, ts(mi, P)],
                            rhs=wo_f[:, q, ts(half, 512)],
                            start=(q == 0),
                            stop=(q == 1),
                        )
                    o_sb = o_pool.tile([P, 512], f32, tag="o_sb")
                    nc.vector.tensor_copy(o_sb[:], ps[:])
                    nc.gpsimd.dma_start(out_d[ts(mi, P), ts(half, 512)], o_sb[:])

    nc.finalize()
    return nc


def _get_nc():
    global _nc_cache
    if _nc_cache is None:
        _nc_cache = _build_bass()
    return _nc_cache


def kernel(**inputs) -> np.ndarray:
    global LAST_RESULTS
    from concourse.bass_utils import run_bass_kernel_spmd

    x = np.ascontiguousarray(
        np.asarray(inputs["x"], dtype=np.float32).reshape(M_TOTAL, D_MODEL)
    )
    wv = np.ascontiguousarray(np.asarray(inputs["Wv"], dtype=np.float32))
    wo = np.ascontiguousarray(np.asarray(inputs["Wo"], dtype=np.float32))

    nc = _get_nc()
    in_maps = [
        {"x": x[i * M_CORE : (i + 1) * M_CORE], "wv": wv, "wo": wo}
        for i in range(N_CORES)
    ]
    res = run_bass_kernel_spmd(
        nc,
        in_maps,
        list(range(N_CORES)),
        trace=TRACE,
        trace_cores=TRACE_CORES,
    )
    LAST_RESULTS = res
    out = np.concatenate([r["out"] for r in res.results], axis=0)
    return out.reshape(B, T, D_MODEL)


# revision 15
# speedup vs baseline: 1.3343x; 1.3343x over previous
"""Bass/Tile kernel for nn_MicrotubuleAttention on 8 Trainium2 NeuronCores.

Math: the reference adds (1 - gtp) * NEG (NEG = -1e9) to every causal
off-diagonal score. With gamma clipped to >= 1e-4, the smallest penalty is
-1e9 * (1 - exp(-1e-4)) ~= -1e5, so after float32 softmax (max-subtract +
exp) every off-diagonal weight underflows to exactly 0 and attention is
exactly the identity. Hence:

    out = repeat_gqa(x @ Wv) @ Wo = (x @ Wv) @ Wo_folded

where Wo_folded[c*64+d, :] = sum_r Wo[(4c+r)*64+d, :] sums the 4 query-head
row blocks that share KV head c. Q/K/RoPE/polarity/gamma provably do not
affect the f32 output (verified ~1e-6 max rel err against the jax reference).

Sharding: data parallel over rows. B*T = 4096 rows split 8 ways -> 512 rows
per core; Wv/Wo broadcast. Per core, pipelined per 128-row chunk mi:
  1. transpose chunk mi via PE (fp32 has no DMA transpose)
  2. stage 1 for chunk mi:  vT[j', mi] = Wv[:, j']^T @ xT[:, :, mi]
  3. (in parallel) fold Wo: pair adds on DVE, 64-partition shift via
     4 independent SBUF->SBUF DMAs, final adds on DVE -> WoF [256, 1024]
  4. stage 2:  out[mi, n] = vT[:, mi]^T @ WoF

DMA priority: x gates the PE pipeline, Wv gates stage 1, Wo only gates
stage 2, so the Wo loads are chained behind the last x chunk.
"""

import os
import sys

import numpy as np

for _p in ("/opt/trn_rl_repo", "/opt/pypackages"):
    if os.path.isdir(_p) and _p not in sys.path:
        sys.path.append(_p)

B, T, D_MODEL = 2, 2048, 1024
H_Q, H_KV, D_HEAD = 16, 4, 64
N_CORES = 8
M_TOTAL = B * T              # 4096 rows
M_CORE = M_TOTAL // N_CORES  # 512 rows per core
P = 128
KK = D_MODEL // P            # 8 contraction chunks of 128
MC = M_CORE // P             # 4 row chunks of 128
NKV = H_KV * D_HEAD          # 256

TRACE = False          # test.py flips this to profile
TRACE_CORES = None
LAST_RESULTS = None    # BassKernelResults of the most recent run

_nc_cache = None


def _build_bass():
    import concourse.bass as bass
    import concourse.mybir as mybir
    import concourse.tile as tile
    from concourse import bacc
    from concourse.masks import make_identity
    from concourse.tile import add_dep_helper

    f32 = mybir.dt.float32
    ts = bass.ts

    nc = bacc.Bacc(None)
    x_d = nc.declare_dram_parameter("x", [M_CORE, D_MODEL], f32, isOutput=False)
    wv_d = nc.declare_dram_parameter("wv", [D_MODEL, NKV], f32, isOutput=False)
    wo_d = nc.declare_dram_parameter("wo", [H_Q * D_HEAD, D_MODEL], f32, isOutput=False)
    out_d = nc.declare_dram_parameter("out", [M_CORE, D_MODEL], f32, isOutput=True)

    with tile.TileContext(nc) as tc:
        with (
            tc.tile_pool(name="const", bufs=1) as const,
            tc.tile_pool(name="wo_pool", bufs=H_KV) as wo_pool,
            tc.tile_pool(name="x_pool", bufs=MC) as x_pool,
            tc.tile_pool(name="o_pool", bufs=2 * MC) as o_pool,
            tc.tile_pool(name="psum_t", bufs=2, space="PSUM") as psum_t,
            tc.tile_pool(name="psum_mm", bufs=2, space="PSUM") as psum_mm,
        ):
            identity = const.tile([P, P], f32)
            make_identity(nc, identity)
            warm = psum_t.tile([P, P], f32, tag="warm")
            nc.tensor.transpose(warm[:], identity[:], identity[:])

            # ---- Wv load on the second HWDGE queue (Activation engine) so
            # it streams in parallel with the x chunks on the sync queue ----
            wv_sb = const.tile([P, KK, NKV], f32)
            wv_dma = nc.scalar.dma_start(
                wv_sb[:], wv_d.rearrange("(ko p) n -> p ko n", p=P)
            )
            xT = const.tile([P, KK, M_CORE], f32)
            vT = const.tile([P, 2, M_CORE], f32)

            # ---- per-chunk: load x, transpose, stage 1 ----
            x_dmas = []
            for mi in range(MC):
                x_sb = x_pool.tile([P, D_MODEL], f32, tag="x_in")
                x_dmas.append(nc.sync.dma_start(x_sb[:], x_d[ts(mi, P), :]))
                # 4 transposes share one PSUM bank -> one coarse DVE copyback
                for g in range(2):
                    pt = psum_t.tile([P, 512], f32, tag="tp")
                    for j in range(4):
                        kk = g * 4 + j
                        nc.tensor.transpose(
                            pt[:, ts(j, P)], x_sb[:, ts(kk, P)], identity[:]
                        )
                    nc.vector.tensor_copy(
                        xT[:, ts(g, 4), ts(mi, P)],
                        pt.rearrange("p (j m) -> p j m", j=4),
                    )
                # stage 1 for this chunk
                for q in range(2):
                    ps = psum_mm.tile([P, P], f32, tag="s1")
                    for kk in range(KK):
                        nc.tensor.matmul(
                            ps[:],
                            lhsT=wv_sb[:, kk, ts(q, P)],
                            rhs=xT[:, kk, ts(mi, P)],
                            start=(kk == 0),
                            stop=(kk == KK - 1),
                        )
                    nc.vector.tensor_copy(vT[:, q, ts(mi, P)], ps[:])

            # ---- Wo: full GQA fold -> WoF[p, q, n], j' = q*128 + p ----
            #   pair_c[p, n] = Wo[256c + p, n] + Wo[256c + 128 + p, n]  (DVE)
            #   shift_c = pair_c[64:128]  (SBUF->SBUF DMA, partition remap)
            #   WoF[(c%2)*64 + d, c//2, n] = pair_c[d, n] + shift_c[d, n] (DVE)
            wo_f = const.tile([P, 2, D_MODEL], f32)
            first_wo = None
            for c in range(H_KV):
                t01 = wo_pool.tile([P, 2, D_MODEL], f32, tag="wo_raw")
                d0 = nc.scalar.dma_start(
                    t01[:],
                    wo_d[256 * c : 256 * (c + 1), :].rearrange(
                        "(two p) n -> p two n", p=P
                    ),
                )
                if first_wo is None:
                    first_wo = d0
                pair = wo_pool.tile([P, D_MODEL], f32, tag="wo_pair")
                nc.vector.tensor_add(pair[:], t01[:, 0, :], t01[:, 1, :])
                shift = wo_pool.tile([64, D_MODEL], f32, tag="wo_shift")
                nc.gpsimd.dma_start(shift[:], pair[64:128, :])
                lo = (c % 2) * 64
                nc.vector.tensor_add(
                    wo_f[lo : lo + 64, c // 2, :], pair[0:64, :], shift[:]
                )
            add_dep_helper(first_wo.ins, x_dmas[-1].ins, reason="x before wo")

            # ---- stage 2: out[m, n] = sum_j' vT[j', m] WoF[j', n] ----
            for mi in range(MC):
                for half in range(2):
                    ps = psum_mm.tile([P, 512], f32, tag="s2")
                    for q in range(2):
                        nc.tensor.matmul(
                            ps[:],
                            lhsT=vT[:, q, ts(mi, P)],
                            rhs=wo_f[:, q, ts(half, 512)],
                            start=(q == 0),
                            stop=(q == 1),
                        )
                    o_sb = o_pool.tile([P, 512], f32, tag="o_sb")
                    nc.vector.tensor_copy(o_sb[:], ps[:])
                    nc.gpsimd.dma_start(out_d[ts(mi, P), ts(half, 512)], o_sb[:])

    nc.finalize()
    return nc


def _get_nc():
    global _nc_cache
    if _nc_cache is None:
        _nc_cache = _build_bass()
    return _nc_cache


def kernel(**inputs) -> np.ndarray:
    global LAST_RESULTS
    from concourse.bass_utils import run_bass_kernel_spmd

    x = np.ascontiguousarray(
        np.asarray(inputs["x"], dtype=np.float32).reshape(M_TOTAL, D_MODEL)
    )
    wv = np.ascontiguousarray(np.asarray(inputs["Wv"], dtype=np.float32))
    wo = np.ascontiguousarray(np.asarray(inputs["Wo"], dtype=np.float32))

    nc = _get_nc()
    in_maps = [
        {"x": x[i * M_CORE : (i + 1) * M_CORE], "wv": wv, "wo": wo}
        for i in range(N_CORES)
    ]
    res = run_bass_kernel_spmd(
        nc,
        in_maps,
        list(range(N_CORES)),
        trace=TRACE,
        trace_cores=TRACE_CORES,
    )
    LAST_RESULTS = res
    out = np.concatenate([r["out"] for r in res.results], axis=0)
    return out.reshape(B, T, D_MODEL)


# revision 16
# speedup vs baseline: 1.3404x; 1.0046x over previous
"""Bass/Tile kernel for nn_MicrotubuleAttention on 8 Trainium2 NeuronCores.

Math: the reference adds (1 - gtp) * NEG (NEG = -1e9) to every causal
off-diagonal score. With gamma clipped to >= 1e-4, the smallest penalty is
-1e9 * (1 - exp(-1e-4)) ~= -1e5, so after float32 softmax (max-subtract +
exp) every off-diagonal weight underflows to exactly 0 and attention is
exactly the identity. Hence:

    out = repeat_gqa(x @ Wv) @ Wo = (x @ Wv) @ Wo_folded

where Wo_folded[c*64+d, :] = sum_r Wo[(4c+r)*64+d, :] sums the 4 query-head
row blocks that share KV head c. Q/K/RoPE/polarity/gamma provably do not
affect the f32 output (verified ~1e-6 max rel err against the jax reference).

Sharding: data parallel over rows. B*T = 4096 rows split 8 ways -> 512 rows
per core; Wv/Wo broadcast. Per core, pipelined per 128-row chunk mi:
  1. transpose chunk mi via PE (fp32 has no DMA transpose)
  2. stage 1 for chunk mi:  vT[j', mi] = Wv[:, j']^T @ xT[:, :, mi]
  3. (in parallel) fold Wo: pair adds on DVE, 64-partition shift via
     4 independent SBUF->SBUF DMAs, final adds on DVE -> WoF [256, 1024]
  4. stage 2:  out[mi, n] = vT[:, mi]^T @ WoF

DMA priority: x gates the PE pipeline, Wv gates stage 1, Wo only gates
stage 2, so the Wo loads are chained behind the last x chunk.
"""

import os
import sys

import numpy as np

for _p in ("/opt/trn_rl_repo", "/opt/pypackages"):
    if os.path.isdir(_p) and _p not in sys.path:
        sys.path.append(_p)

B, T, D_MODEL = 2, 2048, 1024
H_Q, H_KV, D_HEAD = 16, 4, 64
N_CORES = 8
M_TOTAL = B * T              # 4096 rows
M_CORE = M_TOTAL // N_CORES  # 512 rows per core
P = 128
KK = D_MODEL // P            # 8 contraction chunks of 128
MC = M_CORE // P             # 4 row chunks of 128
NKV = H_KV * D_HEAD          # 256

TRACE = False          # test.py flips this to profile
TRACE_CORES = None
LAST_RESULTS = None    # BassKernelResults of the most recent run

_nc_cache = None


def _build_bass():
    import concourse.bass as bass
    import concourse.mybir as mybir
    import concourse.tile as tile
    from concourse import bacc
    from concourse.masks import make_identity
    from concourse.tile import add_dep_helper

    f32 = mybir.dt.float32
    ts = bass.ts

    nc = bacc.Bacc(None)
    x_d = nc.declare_dram_parameter("x", [M_CORE, D_MODEL], f32, isOutput=False)
    wv_d = nc.declare_dram_parameter("wv", [D_MODEL, NKV], f32, isOutput=False)
    wo_d = nc.declare_dram_parameter("wo", [H_Q * D_HEAD, D_MODEL], f32, isOutput=False)
    out_d = nc.declare_dram_parameter("out", [M_CORE, D_MODEL], f32, isOutput=True)

    with tile.TileContext(nc) as tc:
        with (
            tc.tile_pool(name="const", bufs=1) as const,
            tc.tile_pool(name="wo_pool", bufs=H_KV) as wo_pool,
            tc.tile_pool(name="x_pool", bufs=MC) as x_pool,
            tc.tile_pool(name="o_pool", bufs=2 * MC) as o_pool,
            tc.tile_pool(name="psum_t", bufs=2, space="PSUM") as psum_t,
            tc.tile_pool(name="psum_mm", bufs=2, space="PSUM") as psum_mm,
        ):
            identity = const.tile([P, P], f32)
            make_identity(nc, identity)
            warm = psum_t.tile([P, P], f32, tag="warm")
            nc.tensor.transpose(warm[:], identity[:], identity[:])

            # ---- Wv load on the second HWDGE queue (Activation engine) so
            # it streams in parallel with the x chunks on the sync queue ----
            wv_sb = const.tile([P, KK, NKV], f32)
            wv_dma = nc.scalar.dma_start(
                wv_sb[:], wv_d.rearrange("(ko p) n -> p ko n", p=P)
            )
            xT = const.tile([P, KK, M_CORE], f32)
            vT = const.tile([P, 2, M_CORE], f32)

            # ---- per-chunk pipeline: transposes run PIPE chunks ahead of
            # stage 1 so early stage-1 work never stalls on the (slow,
            # strided) Wv load; x halves load separately for finer arrival.
            def emit_transpose(mi, x_sb):
                for g in range(2):
                    pt = psum_t.tile([P, 512], f32, tag="tp")
                    for j in range(4):
                        kk = g * 4 + j
                        nc.tensor.transpose(
                            pt[:, ts(j, P)], x_sb[:, g, ts(j, P)], identity[:]
                        )
                    nc.vector.tensor_copy(
                        xT[:, ts(g, 4), ts(mi, P)],
                        pt.rearrange("p (j m) -> p j m", j=4),
                    )

            def emit_stage1(mi):
                for q in range(2):
                    ps = psum_mm.tile([P, P], f32, tag="s1")
                    for kk in range(KK):
                        nc.tensor.matmul(
                            ps[:],
                            lhsT=wv_sb[:, kk, ts(q, P)],
                            rhs=xT[:, kk, ts(mi, P)],
                            start=(kk == 0),
                            stop=(kk == KK - 1),
                        )
                    nc.vector.tensor_copy(vT[:, q, ts(mi, P)], ps[:])

            PIPE = 2
            x_dmas = []
            xv = x_d.rearrange("m (g n) -> m g n", g=2)
            for mi in range(MC):
                x_sb = x_pool.tile([P, 2, 512], f32, tag="x_in")
                for g in range(2):
                    x_dmas.append(
                        nc.sync.dma_start(x_sb[:, g, :], xv[ts(mi, P), g, :])
                    )
                emit_transpose(mi, x_sb)
                if mi >= PIPE:
                    emit_stage1(mi - PIPE)
            for mi in range(MC - PIPE, MC):
                emit_stage1(mi)

            # ---- Wo: full GQA fold -> WoF[p, q, n], j' = q*128 + p ----
            #   pair_c[p, n] = Wo[256c + p, n] + Wo[256c + 128 + p, n]  (DVE)
            #   shift_c = pair_c[64:128]  (SBUF->SBUF DMA, partition remap)
            #   WoF[(c%2)*64 + d, c//2, n] = pair_c[d, n] + shift_c[d, n] (DVE)
            wo_f = const.tile([P, 2, D_MODEL], f32)
            first_wo = None
            for c in range(H_KV):
                t01 = wo_pool.tile([P, 2, D_MODEL], f32, tag="wo_raw")
                d0 = nc.scalar.dma_start(
                    t01[:],
                    wo_d[256 * c : 256 * (c + 1), :].rearrange(
                        "(two p) n -> p two n", p=P
                    ),
                )
                if first_wo is None:
                    first_wo = d0
                pair = wo_pool.tile([P, D_MODEL], f32, tag="wo_pair")
                nc.vector.tensor_add(pair[:], t01[:, 0, :], t01[:, 1, :])
                shift = wo_pool.tile([64, D_MODEL], f32, tag="wo_shift")
                nc.gpsimd.dma_start(shift[:], pair[64:128, :])
                lo = (c % 2) * 64
                nc.vector.tensor_add(
                    wo_f[lo : lo + 64, c // 2, :], pair[0:64, :], shift[:]
                )
            add_dep_helper(first_wo.ins, x_dmas[3].ins, reason="x1 before wo")

            # ---- stage 2: out[m, n] = sum_j' vT[j', m] WoF[j', n] ----
            for mi in range(MC):
                for half in range(2):
                    ps = psum_mm.tile([P, 512], f32, tag="s2")
                    for q in range(2):
                        nc.tensor.matmul(
                            ps[:],
                            lhsT=vT[:, q, ts(mi, P)],
                            rhs=wo_f[:, q, ts(half, 512)],
                            start=(q == 0),
                            stop=(q == 1),
                        )
                    o_sb = o_pool.tile([P, 512], f32, tag="o_sb")
                    nc.vector.tensor_copy(o_sb[:], ps[:])
                    nc.gpsimd.dma_start(out_d[ts(mi, P), ts(half, 512)], o_sb[:])

    nc.finalize()
    return nc


def _get_nc():
    global _nc_cache
    if _nc_cache is None:
        _nc_cache = _build_bass()
    return _nc_cache


def kernel(**inputs) -> np.ndarray:
    global LAST_RESULTS
    from concourse.bass_utils import run_bass_kernel_spmd

    x = np.ascontiguousarray(
        np.asarray(inputs["x"], dtype=np.float32).reshape(M_TOTAL, D_MODEL)
    )
    wv = np.ascontiguousarray(np.asarray(inputs["Wv"], dtype=np.float32))
    wo = np.ascontiguousarray(np.asarray(inputs["Wo"], dtype=np.float32))

    nc = _get_nc()
    in_maps = [
        {"x": x[i * M_CORE : (i + 1) * M_CORE], "wv": wv, "wo": wo}
        for i in range(N_CORES)
    ]
    res = run_bass_kernel_spmd(
        nc,
        in_maps,
        list(range(N_CORES)),
        trace=TRACE,
        trace_cores=TRACE_CORES,
    )
    LAST_RESULTS = res
    out = np.concatenate([r["out"] for r in res.results], axis=0)
    return out.reshape(B, T, D_MODEL)


# revision 17
# speedup vs baseline: 1.4569x; 1.0869x over previous
"""Bass/Tile kernel for nn_MicrotubuleAttention on 8 Trainium2 NeuronCores.

Math: the reference adds (1 - gtp) * NEG (NEG = -1e9) to every causal
off-diagonal score. With gamma clipped to >= 1e-4, the smallest penalty is
-1e9 * (1 - exp(-1e-4)) ~= -1e5, so after float32 softmax (max-subtract +
exp) every off-diagonal weight underflows to exactly 0 and attention is
exactly the identity. Hence:

    out = repeat_gqa(x @ Wv) @ Wo = (x @ Wv) @ Wo_folded

where Wo_folded[c*64+d, :] = sum_r Wo[(4c+r)*64+d, :] sums the 4 query-head
row blocks that share KV head c. Q/K/RoPE/polarity/gamma provably do not
affect the f32 output (verified ~1e-6 max rel err against the jax reference).

Sharding: data parallel over rows. B*T = 4096 rows split 8 ways -> 512 rows
per core; Wv/Wo broadcast. Per core, pipelined per 128-row chunk mi:
  1. transpose chunk mi via PE (fp32 has no DMA transpose)
  2. stage 1 for chunk mi:  vT[j', mi] = Wv[:, j']^T @ xT[:, :, mi]
  3. (in parallel) fold Wo: pair adds on DVE, 64-partition shift via
     4 independent SBUF->SBUF DMAs, final adds on DVE -> WoF [256, 1024]
  4. stage 2:  out[mi, n] = vT[:, mi]^T @ WoF

DMA priority: x gates the PE pipeline, Wv gates stage 1, Wo only gates
stage 2, so the Wo loads are chained behind the last x chunk.
"""

import os
import sys

import numpy as np

for _p in ("/opt/trn_rl_repo", "/opt/pypackages"):
    if os.path.isdir(_p) and _p not in sys.path:
        sys.path.append(_p)

B, T, D_MODEL = 2, 2048, 1024
H_Q, H_KV, D_HEAD = 16, 4, 64
N_CORES = 8
M_TOTAL = B * T              # 4096 rows
M_CORE = M_TOTAL // N_CORES  # 512 rows per core
P = 128
KK = D_MODEL // P            # 8 contraction chunks of 128
MC = M_CORE // P             # 4 row chunks of 128
NKV = H_KV * D_HEAD          # 256

TRACE = False          # test.py flips this to profile
TRACE_CORES = None
LAST_RESULTS = None    # BassKernelResults of the most recent run

_nc_cache = None


def _build_bass():
    import concourse.bass as bass
    import concourse.mybir as mybir
    import concourse.tile as tile
    from concourse import bacc
    from concourse.masks import make_identity
    from concourse.tile import add_dep_helper

    f32 = mybir.dt.float32
    ts = bass.ts

    nc = bacc.Bacc(None)
    x_d = nc.declare_dram_parameter("x", [M_CORE, D_MODEL], f32, isOutput=False)
    wv_d = nc.declare_dram_parameter("wv", [D_MODEL, NKV], f32, isOutput=False)
    wo_d = nc.declare_dram_parameter("wo", [H_Q * D_HEAD, D_MODEL], f32, isOutput=False)
    out_d = nc.declare_dram_parameter("out", [M_CORE, D_MODEL], f32, isOutput=True)

    with tile.TileContext(nc) as tc:
        with (
            tc.tile_pool(name="const", bufs=1) as const,
            tc.tile_pool(name="wo_pool", bufs=H_KV) as wo_pool,
            tc.tile_pool(name="x_pool", bufs=MC) as x_pool,
            tc.tile_pool(name="o_pool", bufs=2 * MC) as o_pool,
            tc.tile_pool(name="psum_t", bufs=2, space="PSUM") as psum_t,
            tc.tile_pool(name="psum_mm", bufs=2, space="PSUM") as psum_mm,
        ):
            identity = const.tile([P, P], f32)
            make_identity(nc, identity)
            warm = psum_t.tile([P, P], f32, tag="warm")
            nc.tensor.transpose(warm[:], identity[:], identity[:])

            # ---- Wv load on the second HWDGE queue (Activation engine) so
            # it streams in parallel with the x chunks on the sync queue ----
            wv_sb = const.tile([P, KK, NKV], f32)
            wv_dma = nc.scalar.dma_start(
                wv_sb[:], wv_d.rearrange("(ko p) n -> p ko n", p=P)
            )
            xT = const.tile([P, KK, M_CORE], f32)
            vT = const.tile([P, 2, M_CORE], f32)

            # ---- per-chunk pipeline: transposes run PIPE chunks ahead of
            # stage 1 so early stage-1 work never stalls on the (slow,
            # strided) Wv load; x halves load separately for finer arrival.
            def emit_transpose(mi, x_sb):
                for g in range(2):
                    pt = psum_t.tile([P, 512], f32, tag="tp")
                    for j in range(4):
                        kk = g * 4 + j
                        nc.tensor.transpose(
                            pt[:, ts(j, P)], x_sb[:, g, ts(j, P)], identity[:]
                        )
                    nc.vector.tensor_copy(
                        xT[:, ts(g, 4), ts(mi, P)],
                        pt.rearrange("p (j m) -> p j m", j=4),
                    )

            def emit_stage1(mi):
                for q in range(2):
                    ps = psum_mm.tile([P, P], f32, tag="s1")
                    for kk in range(KK):
                        nc.tensor.matmul(
                            ps[:],
                            lhsT=wv_sb[:, kk, ts(q, P)],
                            rhs=xT[:, kk, ts(mi, P)],
                            start=(kk == 0),
                            stop=(kk == KK - 1),
                        )
                    nc.vector.tensor_copy(vT[:, q, ts(mi, P)], ps[:])

            PIPE = 1
            x_dmas = []
            for mi in range(MC):
                x_sb = x_pool.tile([P, 2, 512], f32, tag="x_in")
                eng = nc.sync if mi % 2 == 0 else nc.scalar
                x_dmas.append(
                    nc.sync.dma_start(
                        x_sb[:], x_d[ts(mi, P), :].rearrange("m (g n) -> m g n", g=2)
                    )
                    if eng is nc.sync
                    else nc.scalar.dma_start(
                        x_sb[:], x_d[ts(mi, P), :].rearrange("m (g n) -> m g n", g=2)
                    )
                )
                emit_transpose(mi, x_sb)
                if mi >= PIPE:
                    emit_stage1(mi - PIPE)
            for mi in range(MC - PIPE, MC):
                emit_stage1(mi)

            # ---- Wo: full GQA fold -> WoF[p, q, n], j' = q*128 + p ----
            #   pair_c[p, n] = Wo[256c + p, n] + Wo[256c + 128 + p, n]  (DVE)
            #   shift_c = pair_c[64:128]  (SBUF->SBUF DMA, partition remap)
            #   WoF[(c%2)*64 + d, c//2, n] = pair_c[d, n] + shift_c[d, n] (DVE)
            wo_f = const.tile([P, 2, D_MODEL], f32)
            first_wo = None
            for c in range(H_KV):
                t01 = wo_pool.tile([P, 2, D_MODEL], f32, tag="wo_raw")
                d0 = nc.scalar.dma_start(
                    t01[:],
                    wo_d[256 * c : 256 * (c + 1), :].rearrange(
                        "(two p) n -> p two n", p=P
                    ),
                )
                if first_wo is None:
                    first_wo = d0
                pair = wo_pool.tile([P, D_MODEL], f32, tag="wo_pair")
                nc.vector.tensor_add(pair[:], t01[:, 0, :], t01[:, 1, :])
                shift = wo_pool.tile([64, D_MODEL], f32, tag="wo_shift")
                nc.gpsimd.dma_start(shift[:], pair[64:128, :])
                lo = (c % 2) * 64
                nc.vector.tensor_add(
                    wo_f[lo : lo + 64, c // 2, :], pair[0:64, :], shift[:]
                )
            add_dep_helper(first_wo.ins, x_dmas[1].ins, reason="x1 before wo")

            # ---- stage 2: out[m, n] = sum_j' vT[j', m] WoF[j', n] ----
            for mi in range(MC):
                for half in range(2):
                    ps = psum_mm.tile([P, 512], f32, tag="s2")
                    for q in range(2):
                        nc.tensor.matmul(
                            ps[:],
                            lhsT=vT[:, q, ts(mi, P)],
                            rhs=wo_f[:, q, ts(half, 512)],
                            start=(q == 0),
                            stop=(q == 1),
                        )
                    o_sb = o_pool.tile([P, 512], f32, tag="o_sb")
                    nc.vector.tensor_copy(o_sb[:], ps[:])
                    nc.gpsimd.dma_start(out_d[ts(mi, P), ts(half, 512)], o_sb[:])

    nc.finalize()
    return nc


def _get_nc():
    global _nc_cache
    if _nc_cache is None:
        _nc_cache = _build_bass()
    return _nc_cache


def kernel(**inputs) -> np.ndarray:
    global LAST_RESULTS
    from concourse.bass_utils import run_bass_kernel_spmd

    x = np.ascontiguousarray(
        np.asarray(inputs["x"], dtype=np.float32).reshape(M_TOTAL, D_MODEL)
    )
    wv = np.ascontiguousarray(np.asarray(inputs["Wv"], dtype=np.float32))
    wo = np.ascontiguousarray(np.asarray(inputs["Wo"], dtype=np.float32))

    nc = _get_nc()
    in_maps = [
        {"x": x[i * M_CORE : (i + 1) * M_CORE], "wv": wv, "wo": wo}
        for i in range(N_CORES)
    ]
    res = run_bass_kernel_spmd(
        nc,
        in_maps,
        list(range(N_CORES)),
        trace=TRACE,
        trace_cores=TRACE_CORES,
    )
    LAST_RESULTS = res
    out = np.concatenate([r["out"] for r in res.results], axis=0)
    return out.reshape(B, T, D_MODEL)
